# revision 46
# baseline (speedup 1.0000x reference)
"""Causal self-attention (B=2, T=2048, D=1024, H=16) on 8 TRN2 NeuronCores.

Sharding: data-parallel over batch (2) x tensor-parallel over head groups (4),
so each core handles one batch element and 4 heads (256 of the 1024 attention
channels). The out-projection is row-sharded; the host sums the 4 partial
outputs per batch element in fp32.

Per-core kernel (single fused pipeline, fp16 matmul inputs, fp32 PSUM):
  - Q^T/K^T in [o, t] layout (lhsT = W slice, rhs = x^T), computed in
    512-token chunks; Q bias applied on VectorE during the PSUM->SBUF copy
  - V in [t, o] layout with a ones column so PV also accumulates the
    softmax denominator l[q] in PSUM row 64
  - attention in the transposed orientation: S^T tiles [128 k, 512 q],
    head pairs on PE base partitions 0/64; exp on ScalarE over [128,1024]
    two-bank groups; causal wedge zeroed by gpsimd affine_select; the
    outermost diagonal group is trimmed to the live query range in the
    QK mms, exp regions, and PV mms
  - schedule: strips interleave head pairs (hp0 s, hp1 s); projection
    chunks and out-projection tiles of completed strips are emitted as
    filler work after each attention group so the PE array stays busy
    while ScalarE paces the exp stream
  - normalization per strip-headpair: l rows -> reciprocal on VectorE,
    DRAM round-trip broadcast (2 DMAs), one tensor_mul
  - out-projection tiles accumulate into [128,512] PSUM halves; outputs
    staged in SBUF and written back with one DMA per 512-token strip
Bias handling: b_k dropped (softmax shift-invariant per query), b_q applied
during the Q copy, b_v and b_out folded into a host-side constant.
"""

import numpy as np

B, T_FULL, D, H = 2, 2048, 1024, 16
DH = 64
HC = 4            # heads per core
OC = HC * DH      # 256 attention channels per core
NCORES = 8


def build_nc(T=T_FULL):
    import concourse.bass as bass
    import concourse.mybir as mybir
    from concourse import bacc
    from concourse.tile import TileContext

    f32 = mybir.dt.float32
    f32r = mybir.dt.float32r
    fp16 = mybir.dt.float16
    AF = mybir.ActivationFunctionType
    ALU = mybir.AluOpType

    KD = D // 128           # contraction tiles for the projections
    TT = T // 128           # token tiles
    NS = T // 512           # query strips / token chunks of 512
    KO = OC // 128          # o-tiles for Q/K (and out-proj contraction)

    # Host pre-arranges every input so each DMA is one flat transfer with
    # >=4KB contiguous runs per partition (near-peak HBM rate).
    nc = bacc.Bacc("TRN2", target_bir_lowering=False)
    NS_ = T // 512
    xT_d = nc.dram_tensor("xT", [128, NS_, D // 128, 512], fp16,
                          kind="ExternalInput")
    wq_d = nc.dram_tensor("wq", [128, D // 128, OC], fp16, kind="ExternalInput")
    wk_d = nc.dram_tensor("wk", [128, D // 128, OC], fp16, kind="ExternalInput")
    wv_d = nc.dram_tensor("wv", [128, D // 128, OC], fp16, kind="ExternalInput")
    bq_d = nc.dram_tensor("bq", [128, OC // 128], f32, kind="ExternalInput")
    wo_d = nc.dram_tensor("wo", [128, OC // 128, D], fp16, kind="ExternalInput")
    out_d = nc.dram_tensor("out", [T, D], fp16, kind="ExternalOutput")

    def mm(out, lhsT, rhs, start, stop):
        if lhsT.dtype == f32:
            lhsT = lhsT.bitcast(f32r)
        if rhs.dtype == f32:
            rhs = rhs.bitcast(f32r)
        nc.tensor.matmul(out, lhsT, rhs, start=start, stop=stop)

    with TileContext(nc) as tc:
        with (
            tc.tile_pool(name="persist", bufs=1) as P1,
            tc.tile_pool(name="work", bufs=3) as WK,
            tc.tile_pool(name="late", bufs=3) as LP,
            # PSUM budget (8 banks): 3x[128,1024] rotation shared by S^T
            # groups / projection chunks / V groups / out-proj tiles
            # (6 banks) + PV accumulators 2x[128,512] (2 banks; the
            # normalization broadcast reuses the dead PV tiles).
            tc.tile_pool(name="pss", bufs=3, space="PSUM") as PSS,
            tc.tile_pool(name="pso", bufs=1, space="PSUM") as PSO,
        ):
            QT = P1.tile([128, KO, T], fp16)
            KT = P1.tile([128, KO, T], fp16)
            V = P1.tile([128, TT, HC, DH + 1], fp16)
            attnT = P1.tile([128, KO, T], fp16)
            wo = P1.tile([128, KO, D], fp16)
            wq = P1.tile([128, KD, OC], fp16)
            wk = P1.tile([128, KD, OC], fp16)
            wv = P1.tile([128, KD, OC], fp16)
            bq = P1.tile([128, KO], f32)
            xT = P1.tile([128, NS, KD, 512], fp16)
            out_sb = P1.tile([128, NS, D], fp16)

            # Batched flat input DMAs, priority order: the first projection
            # chunk needs bq + wq + xT tokens 0:512, then wk + wv. wq and
            # xT chunk 0 are split in k-halves so the first accumulation
            # matmuls can start while the second halves transfer.
            nc.sync.dma_start(bq[:], bq_d[:])
            nc.sync.dma_start(wq[:, 0:4], wq_d[:, 0:4])
            nc.sync.dma_start(xT[:, 0, 0:4], xT_d[:, 0, 0:4])
            nc.sync.dma_start(wq[:, 4:8], wq_d[:, 4:8])
            nc.sync.dma_start(xT[:, 0, 4:8], xT_d[:, 0, 4:8])
            nc.sync.dma_start(wk[:], wk_d[:])
            nc.sync.dma_start(wv[:], wv_d[:])
            for c in range(1, NS):
                nc.sync.dma_start(xT[:, c], xT_d[:, c])
            nc.sync.dma_start(wo[:], wo_d[:])

            ones32 = P1.tile([128, 1], f32)
            nc.gpsimd.memset(ones32[:], 1.0)
            _oap = ones32[:]
            # fp16 ones row for the rank-1 normalization broadcast matmul
            ones16 = P1.tile([1, 128], fp16)
            nc.vector.tensor_copy(
                ones16[:], bass.AP(_oap.tensor, _oap.offset,
                                   [[_oap.ap[0][0], 1], [0, 128]]))
            ones128 = P1.tile([128, 512], fp16)
            nc.vector.tensor_copy(
                ones128[:], bass.AP(_oap.tensor, _oap.offset,
                                    [_oap.ap[0], [0, 512]]))

            def warm_pe(n):
                """Dummy full-array matmuls: keep the PE HAM clock gate
                open (and ramp it at kernel start) while real work waits
                on DMA or the normalization chain. The HAM watches array
                activity, so the dummies must use all 128 rows. Results
                are never read."""
                ds = PSS.tile([128, 1024], f32, tag="ss", name="warm")
                for _ in range(n):
                    mm(ds[:, 0:512], ones128[:, 0:128], ones128[:],
                       start=True, stop=True)

            # ---- emission helpers ----

            def proj_chunk(kind, ot, c):
                """Q or K projection for o-tile ot, tokens [512c, 512c+512)."""
                w_t = wq if kind == "q" else wk
                ps = PSS.tile([128, 1024], f32, tag="ss",
                              name=f"pj{kind}{ot}{c}")[:, 0:512]
                for k in range(KD):
                    mm(ps[:], w_t[:, k, ot * 128:(ot + 1) * 128],
                       xT[:, c, k, :],
                       start=(k == 0), stop=(k == KD - 1))
                if kind == "q":
                    nc.vector.tensor_scalar_add(
                        QT[:, ot, c * 512:(c + 1) * 512], ps[:], bq[:, ot:ot + 1])
                else:
                    nc.vector.tensor_copy(
                        KT[:, ot, c * 512:(c + 1) * 512], ps[:])

            def v_group(tg):
                ps = PSS.tile([128, 1024], f32, tag="ss", name="psv")
                for t4 in range(4):
                    for k in range(KD):
                        mm(ps[:, t4 * 256:(t4 + 1) * 256],
                           xT[:, tg, k, t4 * 128:(t4 + 1) * 128], wv[:, k, :],
                           start=(k == 0), stop=(k == KD - 1))
                nc.vector.tensor_copy(
                    V[:, 4 * tg:4 * tg + 4, :, 0:DH],
                    ps[:].rearrange("p (t h o) -> p t h o", t=4, h=HC))
                # ones column (memset doesn't accept 16-bit dtypes)
                nc.vector.tensor_copy(
                    V[:, 4 * tg:4 * tg + 4, :, DH:DH + 1],
                    bass.AP(_oap.tensor, _oap.offset,
                            [_oap.ap[0], [0, 4], [0, HC], [0, 1]]))

            def outproj_tile(s, i):
                tt = 4 * s + i
                ps = PSS.tile([128, 1024], f32, tag="ss", name=f"op{tt}")
                for k2 in range(KO):
                    for nch in range(2):
                        mm(ps[:, nch * 512:(nch + 1) * 512],
                           attnT[:, k2, tt * 128:(tt + 1) * 128],
                           wo[:, k2, nch * 512:(nch + 1) * 512],
                           start=(k2 == 0), stop=(k2 == KO - 1))
                nc.vector.tensor_copy(out_sb[:, i, :], ps[:])

            def outproj_dma(s):
                nc.sync.dma_start(
                    out_d[s * 512:(s + 1) * 512, :]
                        .rearrange("(i p) n -> p i n", p=128),
                    out_sb[:])

            def attn_strip(hp, s, fillers=(), tail_warm=3):
                heads = (2 * hp, 2 * hp + 1)
                nk = 4 * (s + 1)
                ngroups = nk // 2
                fl = list(fillers)
                fidx = 0

                def filler():
                    nonlocal fidx
                    if fidx < len(fl):
                        fl[fidx]()
                        fidx += 1

                pso = {h: PSO.tile([128, 512], f32, tag=f"po{h % 2}",
                                   name=f"pso{h}")
                       for h in heads}
                for kg in range(ngroups):
                    # last two groups touch the causal diagonal; the very
                    # last one (kil 2,3) is trimmed to the live q range
                    diag1 = kg == ngroups - 2
                    diag2 = kg == ngroups - 1
                    qoffs = []
                    for kk in range(2):
                        kil = 2 * kg + kk - (nk - 4)
                        qoffs.append(128 * kil if diag2 and kil > 0 else 0)
                    pss = {h: PSS.tile([128, 1024], f32, tag="ss",
                                       name=f"pss{h}")
                           for h in heads}
                    for kk in range(2):
                        ki = 2 * kg + kk
                        qo = qoffs[kk]
                        for h in heads:
                            po = (h % 2) * 64
                            mm(pss[h][:, kk * 512 + qo:(kk + 1) * 512],
                               KT[po:po + 64, hp, ki * 128:(ki + 1) * 128],
                               QT[po:po + 64, hp, s * 512 + qo:(s + 1) * 512],
                               start=True, stop=True)
                    filler()
                    pt = {}
                    for h in heads:
                        pt[h] = LP.tile([128, 1024], fp16,
                                        tag=f"pt{h % 2}", name=f"pt{h}")
                        if diag2:
                            for kk in range(2):
                                qo = qoffs[kk]
                                reg = pt[h][:, kk * 512 + qo:(kk + 1) * 512]
                                nc.scalar.activation(
                                    reg,
                                    pss[h][:, kk * 512 + qo:(kk + 1) * 512],
                                    AF.Exp, scale=0.125)
                                nc.gpsimd.affine_select(
                                    reg, reg,
                                    pattern=[[1, 512 - qo]],
                                    compare_op=ALU.is_ge, fill=0.0,
                                    base=0, channel_multiplier=-1)
                        else:
                            nc.scalar.activation(pt[h][:], pss[h][:],
                                                 AF.Exp, scale=0.125)
                    if diag1:
                        # causal wedge for both 512-halves in one call:
                        # keep where q - p - 128*(kil0 + half) >= 0
                        for h in heads:
                            nc.gpsimd.affine_select(
                                pt[h][:].rearrange("p (g q) -> p g q", g=2),
                                pt[h][:].rearrange("p (g q) -> p g q", g=2),
                                pattern=[[-128, 2], [1, 512]],
                                compare_op=ALU.is_ge, fill=0.0,
                                base=0, channel_multiplier=-1)
                    for kk in range(2):
                        ki = 2 * kg + kk
                        qo = qoffs[kk]
                        for h in heads:
                            mm(pso[h][0:DH + 1, qo:512], V[:, ki, h, :],
                               pt[h][:, kk * 512 + qo:(kk + 1) * 512],
                               start=(ki == 0), stop=(ki == nk - 1))
                    filler()
                while fidx < len(fl):
                    fl[fidx]()
                    fidx += 1
                if tail_warm:
                    warm_pe(tail_warm)
                # per-strip epilogue: 1/l = exp(-ln(l)) on ScalarE straight
                # from the PSUM l row (both functions live in the
                # natural_log_exp_and_others table set, so no set switch),
                # then partition-broadcast with a trivial rank-1 matmul
                # (ones[1,64].T @ r_row[1,512] -> dead PV tile) and
                # normalize.
                lt = WK.tile([1, 2, 512], f32, tag="lt")
                rrow = WK.tile([1, 2, 512], fp16, tag="rrow")
                for hi, h in enumerate(heads):
                    nc.scalar.activation(lt[:, hi, :], pso[h][DH:DH + 1, :],
                                         AF.Ln)
                    nc.scalar.activation(rrow[:, hi, :], lt[:, hi, :],
                                         AF.Exp, scale=-1.0)
                for hi, h in enumerate(heads):
                    po = (h % 2) * 64
                    nc.vector.tensor_copy(
                        attnT[po:po + 64, hp, s * 512:(s + 1) * 512],
                        pso[h][0:DH, :])
                # the broadcast matmul reuses the (fully read) PV tile so
                # no extra PSUM bank is needed; odd heads land on
                # partitions 64-127 to line up with their attnT rows.
                for hi, h in enumerate(heads):
                    po = (h % 2) * 64
                    rb = pso[h][po:po + 64, :]
                    mm(rb, ones16[:, 0:64], rrow[:, hi, :],
                       start=True, stop=True)
                    nc.vector.tensor_mul(
                        attnT[po:po + 64, hp, s * 512:(s + 1) * 512],
                        attnT[po:po + 64, hp, s * 512:(s + 1) * 512],
                        rb)

            # ---- emission schedule ----
            # Interleave head pairs per strip; later projection chunks and
            # out-projection tiles of completed strips fill PE bubbles
            # while ScalarE paces the exp stream.
            def P(kind, ot, c):
                return lambda: proj_chunk(kind, ot, c)

            def VG(tg):
                return lambda: v_group(tg)

            def OP(s, i):
                return lambda: outproj_tile(s, i)

            def OPD(s):
                return lambda: outproj_dma(s)

            warm_pe(20)
            proj_chunk("q", 0, 0)
            proj_chunk("k", 0, 0)
            v_group(0)
            attn_strip(0, 0, [P("q", 1, 0), P("k", 1, 0)])
            attn_strip(1, 0, [P("q", 0, 1), P("k", 0, 1), VG(1)])
            attn_strip(0, 1, [P("q", 1, 1), P("k", 1, 1)])
            attn_strip(1, 1, [P("q", 0, 2), P("k", 0, 2), VG(2),
                              OP(0, 0), OP(0, 1)])
            attn_strip(0, 2, [P("q", 1, 2), P("k", 1, 2),
                              OP(0, 2), OP(0, 3), OPD(0)])
            attn_strip(1, 2, [P("q", 0, 3), P("k", 0, 3), VG(3),
                              OP(1, 0), OP(1, 1)])
            attn_strip(0, 3, [P("q", 1, 3), P("k", 1, 3),
                              OP(1, 2), OP(1, 3), OPD(1)])
            attn_strip(1, 3, [OP(2, 0), OP(2, 1), OP(2, 2), OP(2, 3), OPD(2)],
                       tail_warm=24)
            for i in range(4):
                outproj_tile(3, i)
                tt = 12 + i
                nc.sync.dma_start(out_d[tt * 128:(tt + 1) * 128, :],
                                  out_sb[:, i, :])

    if hasattr(nc, "compile"):
        nc.compile()
    return nc


def _wlayout(w):
    """[K*128, N] -> [128, K, N] (partition-major flat for one DMA)."""
    K = w.shape[0] // 128
    return np.ascontiguousarray(
        w.reshape(K, 128, -1).transpose(1, 0, 2).astype(np.float16))


def shard_inputs(x, w_qkv, b_qkv, w_out):
    """Build the 8 per-core input dicts (core = b * 4 + g)."""
    KD, NS = D // 128, T_FULL // 512
    in_maps = []
    xTs = []
    for b in range(B):
        xT = np.asarray(x[b]).T.astype(np.float16)          # [D, T]
        xTs.append(np.ascontiguousarray(
            xT.reshape(KD, 128, NS, 512).transpose(1, 2, 0, 3)))
    for core in range(NCORES):
        b, g = core // 4, core % 4
        o0 = g * OC
        in_maps.append({
            "xT": xTs[b],
            "wq": _wlayout(w_qkv[:, o0:o0 + OC]),
            "wk": _wlayout(w_qkv[:, D + o0:D + o0 + OC]),
            "wv": _wlayout(w_qkv[:, 2 * D + o0:2 * D + o0 + OC]),
            "bq": np.ascontiguousarray(
                b_qkv[o0:o0 + OC].astype(np.float32).reshape(2, 128).T),
            "wo": _wlayout(w_out[o0:o0 + OC, :]),
        })
    return in_maps


_NC_CACHE = {}


def kernel(x, w_qkv, b_qkv, w_out, b_out):
    from concourse.bass_utils import run_bass_kernel_spmd

    x = np.asarray(x, dtype=np.float32)
    w_qkv = np.asarray(w_qkv, dtype=np.float32)
    b_qkv = np.asarray(b_qkv, dtype=np.float32)
    w_out = np.asarray(w_out, dtype=np.float32)
    b_out = np.asarray(b_out, dtype=np.float32)

    if "nc" not in _NC_CACHE:
        _NC_CACHE["nc"] = build_nc(T_FULL)
    nc = _NC_CACHE["nc"]

    in_maps = shard_inputs(x, w_qkv, b_qkv, w_out)
    res = run_bass_kernel_spmd(nc, in_maps, list(range(NCORES)))

    # b_v and b_out folded here: softmax rows sum to 1, so the v-bias
    # contributes b_v @ w_out to every token.
    b_eff = (b_out + b_qkv[2 * D:] @ w_out).astype(np.float32)
    out = np.empty((B, T_FULL, D), dtype=np.float32)
    for b in range(B):
        acc = res.results[b * 4]["out"].astype(np.float32)
        for g in range(1, 4):
            acc = acc + res.results[b * 4 + g]["out"].astype(np.float32)
        out[b] = acc + b_eff
    return out


# revision 53
# speedup vs baseline: 1.2939x; 1.2939x over previous
"""Causal self-attention (B=2, T=2048, D=1024, H=16) on 8 TRN2 NeuronCores.

Sharding: data-parallel over batch (2) x tensor-parallel over head groups (4),
so each core handles one batch element and 4 heads (256 of the 1024 attention
channels). The out-projection is row-sharded; the host sums the 4 partial
outputs per batch element in fp32.

Per-core kernel (single fused pipeline, fp16 matmul inputs, fp32 PSUM):
  - Q^T/K^T in [o, t] layout (lhsT = W slice, rhs = x^T), computed in
    512-token chunks; Q bias applied on VectorE during the PSUM->SBUF copy
  - V in [t, o] layout with a ones column so PV also accumulates the
    softmax denominator l[q] in PSUM row 64
  - attention in the transposed orientation: S^T tiles [128 k, 512 q],
    head pairs on PE base partitions 0/64; exp on ScalarE over [128,1024]
    two-bank groups; causal wedge zeroed by gpsimd affine_select; the
    outermost diagonal group is trimmed to the live query range in the
    QK mms, exp regions, and PV mms
  - schedule: strips interleave head pairs (hp0 s, hp1 s); projection
    chunks and out-projection tiles of completed strips are emitted as
    filler work after each attention group so the PE array stays busy
    while ScalarE paces the exp stream
  - normalization per strip-headpair: l rows -> reciprocal on VectorE,
    DRAM round-trip broadcast (2 DMAs), one tensor_mul
  - out-projection tiles accumulate into [128,512] PSUM halves; outputs
    staged in SBUF and written back with one DMA per 512-token strip
Bias handling: b_k dropped (softmax shift-invariant per query), b_q applied
during the Q copy, b_v and b_out folded into a host-side constant.
"""

import numpy as np

B, T_FULL, D, H = 2, 2048, 1024, 16
DH = 64
HC = 4            # heads per core
OC = HC * DH      # 256 attention channels per core
NCORES = 8


def build_nc(T=T_FULL):
    import concourse.bass as bass
    import concourse.mybir as mybir
    from concourse import bacc
    from concourse.tile import TileContext

    f32 = mybir.dt.float32
    f32r = mybir.dt.float32r
    fp16 = mybir.dt.float16
    AF = mybir.ActivationFunctionType
    ALU = mybir.AluOpType

    KD = D // 128           # contraction tiles for the projections
    TT = T // 128           # token tiles
    NS = T // 512           # query strips / token chunks of 512
    KO = OC // 128          # o-tiles for Q/K (and out-proj contraction)

    # Host pre-arranges every input so each DMA is one flat transfer with
    # >=4KB contiguous runs per partition (near-peak HBM rate).
    nc = bacc.Bacc("TRN2", target_bir_lowering=False)
    NS_ = T // 512
    xT_d = nc.dram_tensor("xT", [128, NS_, D // 128, 512], fp16,
                          kind="ExternalInput")
    wq_d = nc.dram_tensor("wq", [128, D // 128, OC], fp16, kind="ExternalInput")
    wk_d = nc.dram_tensor("wk", [128, D // 128, OC], fp16, kind="ExternalInput")
    wv_d = nc.dram_tensor("wv", [128, D // 128, OC], fp16, kind="ExternalInput")
    bq_d = nc.dram_tensor("bq", [128, OC // 128], f32, kind="ExternalInput")
    wo_d = nc.dram_tensor("wo", [128, OC // 128, D], fp16, kind="ExternalInput")
    out_d = nc.dram_tensor("out", [T, D], fp16, kind="ExternalOutput")

    def mm(out, lhsT, rhs, start, stop):
        if lhsT.dtype == f32:
            lhsT = lhsT.bitcast(f32r)
        if rhs.dtype == f32:
            rhs = rhs.bitcast(f32r)
        nc.tensor.matmul(out, lhsT, rhs, start=start, stop=stop)

    with TileContext(nc) as tc:
        with (
            tc.tile_pool(name="persist", bufs=1) as P1,
            tc.tile_pool(name="work", bufs=3) as WK,
            tc.tile_pool(name="late", bufs=3) as LP,
            # PSUM budget (8 banks): 3x[128,1024] rotation shared by S^T
            # groups / projection chunks / V groups / out-proj tiles
            # (6 banks) + PV accumulators 2x[128,512] (2 banks; the
            # normalization broadcast reuses the dead PV tiles).
            tc.tile_pool(name="pss", bufs=3, space="PSUM") as PSS,
            tc.tile_pool(name="pso", bufs=1, space="PSUM") as PSO,
        ):
            QT = P1.tile([128, KO, T], fp16)
            KT = P1.tile([128, KO, T], fp16)
            V = P1.tile([128, TT, HC, DH + 1], fp16)
            attnT = P1.tile([128, KO, T], fp16)
            wo = P1.tile([128, KO, D], fp16)
            wq = P1.tile([128, KD, OC], fp16)
            wk = P1.tile([128, KD, OC], fp16)
            wv = P1.tile([128, KD, OC], fp16)
            bq = P1.tile([128, KO], f32)
            xT = P1.tile([128, NS, KD, 512], fp16)
            out_sb = P1.tile([128, NS, D], fp16)

            # Batched flat input DMAs, priority order: the first projection
            # chunk needs bq + wq + xT tokens 0:512, then wk + wv. wq and
            # xT chunk 0 are split in k-halves so the first accumulation
            # matmuls can start while the second halves transfer.
            nc.sync.dma_start(bq[:], bq_d[:])
            nc.sync.dma_start(wq[:, 0:4], wq_d[:, 0:4])
            nc.sync.dma_start(xT[:, 0, 0:4], xT_d[:, 0, 0:4])
            nc.sync.dma_start(wq[:, 4:8], wq_d[:, 4:8])
            nc.sync.dma_start(xT[:, 0, 4:8], xT_d[:, 0, 4:8])
            nc.sync.dma_start(wk[:], wk_d[:])
            nc.sync.dma_start(wv[:], wv_d[:])
            for c in range(1, NS):
                nc.sync.dma_start(xT[:, c], xT_d[:, c])
            nc.sync.dma_start(wo[:], wo_d[:])

            ones32 = P1.tile([128, 1], f32)
            nc.gpsimd.memset(ones32[:], 1.0)
            _oap = ones32[:]
            # fp16 ones row for the rank-1 normalization broadcast matmul
            ones16 = P1.tile([1, 128], fp16)
            nc.vector.tensor_copy(
                ones16[:], bass.AP(_oap.tensor, _oap.offset,
                                   [[_oap.ap[0][0], 1], [0, 128]]))
            ones128 = P1.tile([128, 512], fp16)
            nc.vector.tensor_copy(
                ones128[:], bass.AP(_oap.tensor, _oap.offset,
                                    [_oap.ap[0], [0, 512]]))
            ones64f = P1.tile([1, 64], f32)
            nc.gpsimd.memset(ones64f[:], 1.0)

            def warm_pe(n):
                """Dummy full-array matmuls: keep the PE HAM clock gate
                open (and ramp it at kernel start) while real work waits
                on DMA or the normalization chain. The HAM watches array
                activity, so the dummies must use all 128 rows. Results
                are never read."""
                ds = PSS.tile([128, 1024], f32, tag="ss", name="warm")
                for _ in range(n):
                    mm(ds[:, 0:512], ones128[:, 0:128], ones128[:],
                       start=True, stop=True)

            # ---- emission helpers ----

            def proj_chunk(kind, ot, c):
                """Q or K projection for o-tile ot, tokens [512c, 512c+512)."""
                w_t = wq if kind == "q" else wk
                ps = PSS.tile([128, 1024], f32, tag="ss",
                              name=f"pj{kind}{ot}{c}")[:, 0:512]
                for k in range(KD):
                    mm(ps[:], w_t[:, k, ot * 128:(ot + 1) * 128],
                       xT[:, c, k, :],
                       start=(k == 0), stop=(k == KD - 1))
                if kind == "q":
                    nc.vector.tensor_scalar_add(
                        QT[:, ot, c * 512:(c + 1) * 512], ps[:], bq[:, ot:ot + 1])
                else:
                    nc.vector.tensor_copy(
                        KT[:, ot, c * 512:(c + 1) * 512], ps[:])

            def v_group(tg):
                ps = PSS.tile([128, 1024], f32, tag="ss", name="psv")
                for t4 in range(4):
                    for k in range(KD):
                        mm(ps[:, t4 * 256:(t4 + 1) * 256],
                           xT[:, tg, k, t4 * 128:(t4 + 1) * 128], wv[:, k, :],
                           start=(k == 0), stop=(k == KD - 1))
                nc.vector.tensor_copy(
                    V[:, 4 * tg:4 * tg + 4, :, 0:DH],
                    ps[:].rearrange("p (t h o) -> p t h o", t=4, h=HC))
                # ones column (memset doesn't accept 16-bit dtypes)
                nc.vector.tensor_copy(
                    V[:, 4 * tg:4 * tg + 4, :, DH:DH + 1],
                    bass.AP(_oap.tensor, _oap.offset,
                            [_oap.ap[0], [0, 4], [0, HC], [0, 1]]))

            def outproj_tile(s, i):
                tt = 4 * s + i
                ps = PSS.tile([128, 1024], f32, tag="ss", name=f"op{tt}")
                for k2 in range(KO):
                    for nch in range(2):
                        mm(ps[:, nch * 512:(nch + 1) * 512],
                           attnT[:, k2, tt * 128:(tt + 1) * 128],
                           wo[:, k2, nch * 512:(nch + 1) * 512],
                           start=(k2 == 0), stop=(k2 == KO - 1))
                nc.vector.tensor_copy(out_sb[:, i, :], ps[:])

            def outproj_dma(s):
                nc.sync.dma_start(
                    out_d[s * 512:(s + 1) * 512, :]
                        .rearrange("(i p) n -> p i n", p=128),
                    out_sb[:])

            def attn_strip(hp, s, fillers=(), tail_warm=3):
                heads = (2 * hp, 2 * hp + 1)
                nk = 4 * (s + 1)
                ngroups = nk // 2
                fl = list(fillers)
                fidx = 0

                def filler():
                    nonlocal fidx
                    if fidx < len(fl):
                        fl[fidx]()
                        fidx += 1

                pso = {h: PSO.tile([128, 512], f32, tag=f"po{h % 2}",
                                   name=f"pso{h}")
                       for h in heads}
                for kg in range(ngroups):
                    # last two groups touch the causal diagonal; the very
                    # last one (kil 2,3) is trimmed to the live q range
                    diag1 = kg == ngroups - 2
                    diag2 = kg == ngroups - 1
                    qoffs = []
                    for kk in range(2):
                        kil = 2 * kg + kk - (nk - 4)
                        qoffs.append(128 * kil if diag2 and kil > 0 else 0)
                    pss = {h: PSS.tile([128, 1024], f32, tag="ss",
                                       name=f"pss{h}")
                           for h in heads}
                    for kk in range(2):
                        ki = 2 * kg + kk
                        qo = qoffs[kk]
                        for h in heads:
                            po = (h % 2) * 64
                            mm(pss[h][:, kk * 512 + qo:(kk + 1) * 512],
                               KT[po:po + 64, hp, ki * 128:(ki + 1) * 128],
                               QT[po:po + 64, hp, s * 512 + qo:(s + 1) * 512],
                               start=True, stop=True)
                    filler()
                    pt = {}
                    for h in heads:
                        pt[h] = LP.tile([128, 1024], fp16,
                                        tag=f"pt{h % 2}", name=f"pt{h}")
                        if diag2:
                            for kk in range(2):
                                qo = qoffs[kk]
                                reg = pt[h][:, kk * 512 + qo:(kk + 1) * 512]
                                nc.scalar.activation(
                                    reg,
                                    pss[h][:, kk * 512 + qo:(kk + 1) * 512],
                                    AF.Exp, scale=0.125)
                                nc.gpsimd.affine_select(
                                    reg, reg,
                                    pattern=[[1, 512 - qo]],
                                    compare_op=ALU.is_ge, fill=0.0,
                                    base=0, channel_multiplier=-1)
                        else:
                            nc.scalar.activation(pt[h][:], pss[h][:],
                                                 AF.Exp, scale=0.125)
                    if diag1:
                        # causal wedge for both 512-halves in one call:
                        # keep where q - p - 128*(kil0 + half) >= 0
                        for h in heads:
                            nc.gpsimd.affine_select(
                                pt[h][:].rearrange("p (g q) -> p g q", g=2),
                                pt[h][:].rearrange("p (g q) -> p g q", g=2),
                                pattern=[[-128, 2], [1, 512]],
                                compare_op=ALU.is_ge, fill=0.0,
                                base=0, channel_multiplier=-1)
                    for kk in range(2):
                        ki = 2 * kg + kk
                        qo = qoffs[kk]
                        for h in heads:
                            mm(pso[h][0:DH + 1, qo:512], V[:, ki, h, :],
                               pt[h][:, kk * 512 + qo:(kk + 1) * 512],
                               start=(ki == 0), stop=(ki == nk - 1))
                    filler()
                while fidx < len(fl):
                    fl[fidx]()
                    fidx += 1
                if tail_warm:
                    warm_pe(tail_warm)
                # per-strip epilogue: 1/l via reciprocal_approx_fast (~51
                # ULP, safe: l is in [e^-5, 4e3]) straight from the PSUM l
                # row, then partition-broadcast with a trivial rank-1
                # f32r matmul (ones[1,64].T @ r_row[1,512] -> dead PV
                # tile) and normalize.
                lrow = WK.tile([1, 2, 512], f32, tag="lrow")
                rrow = WK.tile([1, 2, 512], f32, tag="rrow")
                rh = WK.tile([1, 2, 512], fp16, tag="rh")
                for hi, h in enumerate(heads):
                    nc.vector.tensor_copy(lrow[:, hi, :],
                                          pso[h][DH:DH + 1, :])
                    nc.vector.reciprocal_approx_fast(
                        rrow[:, hi, :], lrow[:, hi, :])
                    nc.vector.tensor_copy(rh[:, hi, :], rrow[:, hi, :])
                for hi, h in enumerate(heads):
                    po = (h % 2) * 64
                    nc.vector.tensor_copy(
                        attnT[po:po + 64, hp, s * 512:(s + 1) * 512],
                        pso[h][0:DH, :])
                # the broadcast matmul reuses the (fully read) PV tile so
                # no extra PSUM bank is needed; odd heads land on
                # partitions 64-127 to line up with their attnT rows.
                for hi, h in enumerate(heads):
                    po = (h % 2) * 64
                    rb = pso[h][po:po + 64, :]
                    mm(rb, ones16[:, 0:64], rh[:, hi, :],
                       start=True, stop=True)
                    nc.vector.tensor_mul(
                        attnT[po:po + 64, hp, s * 512:(s + 1) * 512],
                        attnT[po:po + 64, hp, s * 512:(s + 1) * 512],
                        rb)

            # ---- emission schedule ----
            # Interleave head pairs per strip; later projection chunks and
            # out-projection tiles of completed strips fill PE bubbles
            # while ScalarE paces the exp stream.
            def P(kind, ot, c):
                return lambda: proj_chunk(kind, ot, c)

            def VG(tg):
                return lambda: v_group(tg)

            def OP(s, i):
                return lambda: outproj_tile(s, i)

            def OPD(s):
                return lambda: outproj_dma(s)

            warm_pe(20)
            proj_chunk("q", 0, 0)
            proj_chunk("k", 0, 0)
            v_group(0)
            attn_strip(0, 0, [P("q", 1, 0), P("k", 1, 0)])
            attn_strip(1, 0, [P("q", 0, 1), P("k", 0, 1), VG(1)])
            attn_strip(0, 1, [P("q", 1, 1), P("k", 1, 1)])
            attn_strip(1, 1, [P("q", 0, 2), P("k", 0, 2), VG(2),
                              OP(0, 0), OP(0, 1)])
            attn_strip(0, 2, [P("q", 1, 2), P("k", 1, 2),
                              OP(0, 2), OP(0, 3), OPD(0)])
            attn_strip(1, 2, [P("q", 0, 3), P("k", 0, 3), VG(3),
                              OP(1, 0), OP(1, 1)])
            attn_strip(0, 3, [P("q", 1, 3), P("k", 1, 3),
                              OP(1, 2), OP(1, 3), OPD(1)])
            attn_strip(1, 3, [OP(2, 0), OP(2, 1), OP(2, 2), OP(2, 3), OPD(2)],
                       tail_warm=24)
            for i in range(4):
                outproj_tile(3, i)
                tt = 12 + i
                nc.sync.dma_start(out_d[tt * 128:(tt + 1) * 128, :],
                                  out_sb[:, i, :])

    if hasattr(nc, "compile"):
        nc.compile()
    return nc


def _wlayout(w):
    """[K*128, N] -> [128, K, N] (partition-major flat for one DMA)."""
    K = w.shape[0] // 128
    return np.ascontiguousarray(
        w.reshape(K, 128, -1).transpose(1, 0, 2).astype(np.float16))


def shard_inputs(x, w_qkv, b_qkv, w_out):
    """Build the 8 per-core input dicts (core = b * 4 + g)."""
    KD, NS = D // 128, T_FULL // 512
    in_maps = []
    xTs = []
    for b in range(B):
        xT = np.asarray(x[b]).T.astype(np.float16)          # [D, T]
        xTs.append(np.ascontiguousarray(
            xT.reshape(KD, 128, NS, 512).transpose(1, 2, 0, 3)))
    for core in range(NCORES):
        b, g = core // 4, core % 4
        o0 = g * OC
        in_maps.append({
            "xT": xTs[b],
            "wq": _wlayout(w_qkv[:, o0:o0 + OC]),
            "wk": _wlayout(w_qkv[:, D + o0:D + o0 + OC]),
            "wv": _wlayout(w_qkv[:, 2 * D + o0:2 * D + o0 + OC]),
            "bq": np.ascontiguousarray(
                b_qkv[o0:o0 + OC].astype(np.float32).reshape(2, 128).T),
            "wo": _wlayout(w_out[o0:o0 + OC, :]),
        })
    return in_maps


_NC_CACHE = {}


def kernel(x, w_qkv, b_qkv, w_out, b_out):
    from concourse.bass_utils import run_bass_kernel_spmd

    x = np.asarray(x, dtype=np.float32)
    w_qkv = np.asarray(w_qkv, dtype=np.float32)
    b_qkv = np.asarray(b_qkv, dtype=np.float32)
    w_out = np.asarray(w_out, dtype=np.float32)
    b_out = np.asarray(b_out, dtype=np.float32)

    if "nc" not in _NC_CACHE:
        _NC_CACHE["nc"] = build_nc(T_FULL)
    nc = _NC_CACHE["nc"]

    in_maps = shard_inputs(x, w_qkv, b_qkv, w_out)
    res = run_bass_kernel_spmd(nc, in_maps, list(range(NCORES)))

    # b_v and b_out folded here: softmax rows sum to 1, so the v-bias
    # contributes b_v @ w_out to every token.
    b_eff = (b_out + b_qkv[2 * D:] @ w_out).astype(np.float32)
    out = np.empty((B, T_FULL, D), dtype=np.float32)
    for b in range(B):
        acc = res.results[b * 4]["out"].astype(np.float32)
        for g in range(1, 4):
            acc = acc + res.results[b * 4 + g]["out"].astype(np.float32)
        out[b] = acc + b_eff
    return out
